# revision 1
# baseline (speedup 1.0000x reference)
"""Trainium2 Bass kernel for nn_CLNGCN (tiny 8-element GNN block).

Math (verified against the reference to ~6e-9 rel err):
    c = cli[0,0]                                  # [8]
    s = c*conv1_w + conv1_b                       # sigma row
    a = c*conv2_w + conv2_b                       # alpha row
    h1 = mlp1_w1 @ c + mlp1_b1 ; h2 = mlp2_w1 @ c + mlp2_b1     # [32]
    u = mlp1_w2 @ gelu(h1) + mlp1_b2              # cli_ss
    v = mlp2_w2 @ gelu(h2) + mlp2_b2              # cli_mm
    ua = u . a
    M[i,j] = v[i]*(ua*a[j]) + (v[i]*s[i])*(u[j]*s[j])           # rank-2
    E = exp(M)  (softmax over i without max-subtraction; |M| < 6)
    seg = relu(c*gcn1_w + gcn1_b)
    out = relu((seg @ E / colsum(E)) * gcn2_w + gcn2_b) + seg   # [1,8]

Device mapping (single core, replicated on 8 cores):
  - One DMA loads a host-packed [66,F] f32 constant/input block.
  - PE matmuls: layer1 (bias folded as K-row), seg-affine column,
    layer2 (biases folded as K-rows), rank-2 M build, fused
    colsum+segdot reduction.
  - ACT: 4x Gelu + 1x Tanh (exp via (1+t)/(1-t)) -- one table set,
    no mid-kernel ACT table reload.
  - DVE: small row ops; relu via max; reciprocal for divisions.
  HW rule: every compute-engine AP (SBUF and PSUM) must start at
  partition 0/32/64/96. Matmul outputs are laid out so each row that
  is consumed individually lands on such a boundary:
    layer2 out rows: [v@0, v@1, u@32, a-slot@64, u@65]
    reduction out:   [colsum@0, segdot@32]
"""

import numpy as np

import concourse.bass as bass
import concourse.tile as tile
from concourse import bacc, mybir
from concourse.bass_utils import run_bass_kernel_spmd

f32 = mybir.dt.float32
AF = mybir.ActivationFunctionType
ALU = mybir.AluOpType

N_CORES = 8

# single-pass PE matmuls (TF32-like). Left off: the BIR verifier requires
# every producer feeding an fp32r matmul to round its output to fp32r,
# which would touch DMA and every ACT/DVE producer for ~0.7us gain.
USE_F32R = False


def _mm(nc, out, lhsT, rhs):
    if USE_F32R:
        lhsT = lhsT.bitcast(mybir.dt.float32r)
        rhs = rhs.bitcast(mybir.dt.float32r)
    nc.tensor.matmul(out, lhsT, rhs)

# column layout of the packed block
C_W2 = 0          # [66,8]  W2stack: mlp1_w2.T | mlp2_w2.T | mlp2_b2 | mlp1_b2
C_W1N = 8         # [64,9]  W1 natural layout: rows=hidden, cols=[W1 | b1]
C_C9 = 17         # [64,9]  c replicated per hidden row, col 8 = 1.0
C_L3 = 73         # [68,66] layer-2 stationary; cols 0,1 <- gelu(h2) (v,v),
                  #         col 32 & 65 <- gelu(h1) (u);
                  #         col 64 computes a = c*conv2_w + conv2_b via
                  #         K-rows 66 (c) / 67 (ones) of the moving block;
                  #         row 64 = b2-enable for v cols, row 65 for u cols
C_Z = C_L3 + 2    # an always-zero column (ACT bias operand)
C_CONES = 139     # [2,8]   row0=c, row1=ones        (seg-affine stationary)
C_GWB = 147       # [2,1]   [gcn1_w; gcn1_b]         (seg-affine moving)
C_C2 = 148        # [2,8]   row0=c, row1=c           (input for X3)
C_X3S1 = 156      # [2,1]   [0; conv1_w]
C_X3S2 = 157      # [2,1]   [1; conv1_b]
C_W2C = 158       # conv2_w
C_B2C = 159       # conv2_b
C_GW1 = 160       # gcn1_w
C_GB1 = 161       # gcn1_b
C_GW2 = 162       # gcn2_w
C_GB2 = 163       # gcn2_b
C_SC2 = 164       # [2,1]   row0 = ua (device-written), row1 = 1.0
C_L5 = 165        # [8,33]  reduction stationary: col0 = ones -> colsum@0,
                  #         col32 = seg (device-written) -> segdot@32
F = 200


def _pack(inputs):
    g = lambda k: np.asarray(inputs[k], np.float32)
    c = g("cli").reshape(8)
    P = np.zeros((68, F), np.float32)
    P[0:32, C_W2:C_W2 + 8] = g("mlp1_w2").T
    P[32:64, C_W2:C_W2 + 8] = g("mlp2_w2").T
    P[64, C_W2:C_W2 + 8] = g("mlp2_b2")
    P[65, C_W2:C_W2 + 8] = g("mlp1_b2")
    P[66, C_W2:C_W2 + 8] = c        # K-row for a = c*conv2_w + conv2_b
    P[67, C_W2:C_W2 + 8] = 1.0
    P[0:32, C_W1N:C_W1N + 8] = g("mlp1_w1")
    P[0:32, C_W1N + 8] = g("mlp1_b1")
    P[32:64, C_W1N:C_W1N + 8] = g("mlp2_w1")
    P[32:64, C_W1N + 8] = g("mlp2_b1")
    P[0:64, C_C9:C_C9 + 8] = c[None, :]
    P[0:64, C_C9 + 8] = 1.0
    # layer-2 stationary bias-enable rows (gelu outputs written on-device)
    P[64, C_L3 + 0] = 1.0    # v col 0 gets mlp2_b2
    P[64, C_L3 + 1] = 1.0    # v col 1
    P[65, C_L3 + 32] = 1.0   # u col 32 gets mlp1_b2
    P[65, C_L3 + 65] = 1.0   # u col 65
    P[66, C_L3 + 64] = g("conv2_w")[0]   # a row (out partition 64)
    P[67, C_L3 + 64] = g("conv2_b")[0]
    P[0, C_CONES:C_CONES + 8] = c
    P[1, C_CONES:C_CONES + 8] = 1.0
    P[0, C_GWB] = g("gcn1_w")[0]
    P[1, C_GWB] = g("gcn1_b")[0]
    P[0, C_C2:C_C2 + 8] = c
    P[1, C_C2:C_C2 + 8] = c
    P[0, C_X3S1] = 0.0
    P[1, C_X3S1] = g("conv1_w")[0]
    P[0, C_X3S2] = 1.0
    P[1, C_X3S2] = g("conv1_b")[0]
    P[0, C_W2C] = g("conv2_w")[0]
    P[0, C_B2C] = g("conv2_b")[0]
    P[0, C_GW1] = g("gcn1_w")[0]
    P[0, C_GB1] = g("gcn1_b")[0]
    P[0, C_GW2] = g("gcn2_w")[0]
    P[0, C_GB2] = g("gcn2_b")[0]
    P[1, C_SC2] = 1.0
    P[0:8, C_L5] = 1.0
    return P


class _LeanTileContext(tile.TileContext):
    """TileContext with a minimal exit: keep the final drain (output DMA
    must land before the NEFF completes) and one barrier, skip the
    semaphore-clear sweep and second barrier. Each kernel() call builds
    and loads a fresh NEFF, so end-state semaphores are never re-entered."""

    def _drain_and_barrier(self, tick_clock, wait_clock):
        drain_inst = self.nc.sync.drain()
        wait_clock.add_sem_waits(
            drain_inst.ins,
            tile.ScopedClock({None: tick_clock.global_clock}),
        )
        assert self.sems is not None
        popped = self.nc._tile_sem_poison_stack.pop()
        assert popped is self._sem_poison


def build(debug=False, lean=True):
    nc = bacc.Bacc("TRN2", target_bir_lowering=False, debug=debug)
    packed = nc.dram_tensor("packed", [68, F], f32, kind="ExternalInput")
    out = nc.dram_tensor("out", [1, 8], f32, kind="ExternalOutput")

    tc_cls = _LeanTileContext if lean else tile.TileContext
    with tc_cls(nc) as tc:
        with (
            tc.tile_pool(name="sb", bufs=1) as sb,
            tc.tile_pool(name="ps", bufs=1, space="PSUM") as ps,
        ):
            big = sb.tile([68, F], f32)
            X3 = sb.tile([2, 8], f32)       # [1; s]
            aRow = sb.tile([1, 8], f32)
            scr = sb.tile([1, 8], f32)
            lhsT4 = sb.tile([2, 8], f32)    # [v; q]
            rhs4 = sb.tile([2, 8], f32)     # [a2; w]
            th = sb.tile([8, 8], f32)       # tanh(M/2)
            num = sb.tile([8, 8], f32)
            rcp8 = sb.tile([8, 8], f32)
            expM = sb.tile([8, 8], f32)
            rcp = sb.tile([1, 8], f32)
            t1 = sb.tile([1, 8], f32)
            t2 = sb.tile([1, 8], f32)
            segR = sb.tile([1, 8], f32)
            fin = sb.tile([1, 8], f32)
            hcol = sb.tile([64, 1], f32)    # h (both MLP hiddens)
            h9 = sb.tile([64, 9], f32)      # elementwise W1*c scratch
            psB = ps.tile([66, 8], f32)     # rows: v@0, v@1, u@32, a@64, u@65
            psC = ps.tile([8, 8], f32)      # M
            psD = ps.tile([8, 1], f32)      # seg affine column
            psE = ps.tile([33, 8], f32)     # colsum@0, segdot@32
            zp = sb.tile([1, 1], f32)       # table-prefetch scratch
            zo = sb.tile([1, 1], f32)

            # Dummy Gelu with no data deps: bacc places the ACT_TABLE_LOAD
            # before the first Gelu in ACT program order, and with no waits
            # ahead of it the ~1.3us gelu_and_others table load runs during
            # the DMA phase instead of stalling the real gelus.
            nc.gpsimd.memset(zp[:, :], 0.0)
            nc.scalar.activation(zo[:, :], zp[:, :], AF.Gelu, bias=zp[:, :])

            # Split input load (all on the Sync HWDGE queue, leaving the
            # Scalar engine free to run its ACT table load immediately):
            # A1: rows 0:64 of W2stack + W1 + replicated-c -- everything
            #     the layer-1 DVE op and (most of) layer-2 need.
            # A2: the constants/rows region used by the small row preps.
            # A3: rows 64:68 -- layer-2 bias/c/ones K-rows + lhsT3 cells.
            # The lhsT3 interior zeros come from memsets, not the DMA.
            nc.sync.dma_start(big[0:64, 0:C_C9 + 9], packed[0:64, 0:C_C9 + 9])
            nc.sync.dma_start(big[0:9, C_CONES:F], packed[0:9, C_CONES:F])
            nc.sync.dma_start(big[64:68, 0:C_L3 + 66],
                              packed[64:68, 0:C_L3 + 66])
            nc.gpsimd.memset(big[0:32, C_L3:C_L3 + 66], 0.0)
            nc.gpsimd.memset(big[32:64, C_L3 + 2:C_L3 + 66], 0.0)

            # layer-1 matvecs for both MLPs on DVE: per-partition dot of
            # W1-row with c (bias as 9th column), accumulated over free dim
            nc.vector.scalar_tensor_tensor(
                h9[:, :], big[0:64, C_W1N:C_W1N + 9], 1.0,
                big[0:64, C_C9:C_C9 + 9], ALU.mult, ALU.mult,
                accum_out=hcol[:, :])

            # independent row preps (DVE), gated on the second DMA
            nc.vector.tensor_scalar(
                X3[:, :], big[0:2, C_C2:C_C2 + 8],
                big[0:2, C_X3S1:C_X3S1 + 1], big[0:2, C_X3S2:C_X3S2 + 1],
                ALU.mult, ALU.add)
            nc.vector.tensor_scalar(
                aRow[:, :], big[0:1, C_CONES:C_CONES + 8],
                big[0:1, C_W2C:C_W2C + 1], big[0:1, C_B2C:C_B2C + 1],
                ALU.mult, ALU.add)

            # PE: seg affine column (K=2)
            _mm(nc, psD[:, :], big[0:2, C_CONES:C_CONES + 8],
                big[0:2, C_GWB:C_GWB + 1])

            # exact GELU on ACT (gelu_and_others table set); duplicate
            # columns via DVE copies to keep the serial ACT chain short
            zb32h = big[32:64, C_Z:C_Z + 1]
            zb32l = big[0:32, C_Z:C_Z + 1]
            nc.scalar.activation(big[32:64, C_L3 + 0:C_L3 + 1], hcol[32:64, :],
                                 AF.Gelu, bias=zb32h)
            nc.scalar.activation(big[0:32, C_L3 + 32:C_L3 + 33], hcol[0:32, :],
                                 AF.Gelu, bias=zb32l)
            # duplicate the gelu columns on DVE, keeping the ACT stream as
            # [load-gelu, gelus, load-exp, Exp] so the exp-set table load
            # hides in ACT idle time instead of delaying anyone's semaphore
            nc.vector.tensor_copy(big[32:64, C_L3 + 1:C_L3 + 2],
                                  big[32:64, C_L3 + 0:C_L3 + 1])
            nc.vector.tensor_copy(big[0:32, C_L3 + 65:C_L3 + 66],
                                  big[0:32, C_L3 + 32:C_L3 + 33])

            # PE: layer 2 -> psB rows [v@0, v@1, u@32, a@64, u@65]
            # (K=68: gelu rows + b2-enable rows + c/ones rows for `a`)
            _mm(nc, psB[0:66, :], big[0:68, C_L3:C_L3 + 66],
                big[0:68, C_W2:C_W2 + 8])

            # ua = sum(u*a) -> big[0, C_SC2]
            nc.vector.scalar_tensor_tensor(
                scr[:, :], aRow[:, :], 1.0, psB[32:33, :],
                ALU.mult, ALU.mult,
                accum_out=big[0:1, C_SC2:C_SC2 + 1])
            # lhsT4 = [v;v]*[1;s] = [v; q]
            nc.vector.tensor_tensor(lhsT4[:, :], psB[0:2, :], X3[:, :], ALU.mult)
            # rhs4 = ([a;u]*[ua;1])*[1;s] = [a2; w]
            nc.vector.scalar_tensor_tensor(
                rhs4[:, :], psB[64:66, :], big[0:2, C_SC2:C_SC2 + 1], X3[:, :],
                ALU.mult, ALU.mult)

            # PE: M = lhsT4.T @ rhs4   [8,8]
            _mm(nc, psC[:, :], lhsT4[:, :], rhs4[:, :])

            # slack ops fill the DVE gap while PE runs M4 and ACT runs Tanh:
            # seg = relu -> reduction stationary col 32 (needed by M5)
            nc.vector.tensor_scalar(big[0:8, C_L5 + 32:C_L5 + 33], psD[:, :],
                                    0.0, None, ALU.max)
            # seg affine row (needed only by the final add)
            nc.vector.tensor_scalar(
                segR[:, :], big[0:1, C_CONES:C_CONES + 8],
                big[0:1, C_GW1:C_GW1 + 1], big[0:1, C_GB1:C_GB1 + 1],
                ALU.mult, ALU.add)

            # exp(M) natively on ACT. The exp_and_others table switch
            # (~1.28us) hides fully in the ACT-idle window between the
            # last gelu copy and M4 completing.
            nc.scalar.activation(expM[:, :], psC[:, :], AF.Exp,
                                 bias=big[0:8, C_Z:C_Z + 1])

            # PE: [colsum@0 ... segdot@32] = L5.T @ expM
            _mm(nc, psE[:, :], big[0:8, C_L5:C_L5 + 33],
                expM[:, :])

            # tail: ss*gw2 = segdot * gw2 / colsum ; relu(+gb2) ; + relu(segR)
            nc.vector.reciprocal(rcp[:, :], psE[0:1, :])
            nc.vector.scalar_tensor_tensor(
                t1[:, :], psE[32:33, :], big[0:1, C_GW2:C_GW2 + 1], rcp[:, :],
                ALU.mult, ALU.mult)
            nc.vector.tensor_scalar(t2[:, :], t1[:, :],
                                    big[0:1, C_GB2:C_GB2 + 1], 0.0,
                                    ALU.add, ALU.max)
            nc.vector.scalar_tensor_tensor(
                fin[:, :], segR[:, :], 0.0, t2[:, :], ALU.max, ALU.add)

            nc.sync.dma_start(out[:, :], fin[:, :])

    # Trim the framework init-block overhead:
    #  - const-AP pool memsets: nothing reads those tensors here;
    #  - the init all-engine barrier + per-engine drains: with the const
    #    memsets gone there is nothing left for them to order (all
    #    kernel-body ordering is carried by Tile's semaphores).
    # Besides executing, these are bir-named instructions, so they would
    # stretch the profiled window by ~2us for no work.
    blk0 = nc.m.functions[0].blocks[0]
    dead = [i for i in blk0.instructions
            if (type(i).__name__ == "InstMemset"
                and i.outs and "const-" in str(getattr(i.outs[0], "memref", "")))
            or type(i).__name__ in ("InstDrain", "InstEventSemaphore")]
    for i in dead:
        blk0.instructions.remove(i)

    nc.compile()

    # Flatten the 3-block CFG (main -> tile body -> end) into one block:
    # the per-engine branch/label pairs are pure overhead for straight-line
    # code, and each engine's instruction order is preserved by simple
    # concatenation.
    f = nc.m.functions[0]
    if len(f.blocks) == 3:
        main, tb, te = f.blocks
        for blk in (main, tb):
            for i in [i for i in blk.instructions
                      if type(i).__name__ == "InstUnconditionalBranch"]:
                blk.instructions.remove(i)
        for i in list(tb.instructions) + list(te.instructions):
            main.instructions.append(i)
        f.blocks.remove(tb)
        f.blocks.remove(te)

    return nc


LAST_RESULTS = None


def kernel(_trace=False, **inputs):
    global LAST_RESULTS
    packed = _pack(inputs)
    nc = build()
    in_maps = [{"packed": packed} for _ in range(N_CORES)]
    res = run_bass_kernel_spmd(nc, in_maps, list(range(N_CORES)), trace=_trace)
    LAST_RESULTS = res
    return res.results[0]["out"]

